# revision 2
# baseline (speedup 1.0000x reference)
"""Trainium2 Bass kernel for CropAndResize (bilinear, TF semantics).

Strategy (8 NeuronCores, SPMD):
  - Shard by image: core k owns image k and the boxes routed to it by
    box_ind (padded to a uniform cap so all cores run one program).
  - Host precomputes gather indices (wrapped for POOL ap_gather) and
    bilinear weights in numpy; they are uploaded as small replicated
    tensors. The device program is just: load image halves into SBUF as
    fp16 adjacent pairs, then per 8-box group: one POOL ap_gather, five
    DVE ops (x-lerp, pair-sum, y-lerp), and a large-descriptor store.
  - The image half (128 channels) lives in SBUF as fp16 *adjacent
    pairs* (imgdup[c, k, :] = (v[k], v[k+1])), so one ap_gather with
    d=2 fetches a bilinear (left, right) pair per index. Out-of-range
    samples are redirected to a zeroed pad slot (index HW), matching
    extrapolation_value=0.0.
  - Gather output order per call: q = ((j*14 + i)*16 + tb*8 + n), d=2
    pairs (l, r) innermost. POOL wraps indices per 16 partitions
    (position q reads the index at partition q%16, slot q//16), so the
    16-row index pattern encodes (tb, n) and is replicated 8x.
  - Output is staged fp16, two 8-box groups per store so each DMA is
    128 descriptors x 6272 B (per-partition contiguous DRAM runs; tiny
    strided descriptors were the baseline's 87 us/iter bottleneck).
    The host reassembles/transposes and upcasts to fp32.
"""

import math

import numpy as np

import concourse.bass as bass
import concourse.bacc as bacc
import concourse.tile as tile
from concourse import mybir
from concourse.bass_utils import run_bass_kernel_spmd

B, C, H, W = 8, 256, 160, 160
CH, CW = 14, 14
HW = H * W  # 25600
N_CORES = 8
CK = 1600  # image-load chunk columns (fp32)

F32 = mybir.dt.float32
F16 = mybir.dt.float16
I16 = mybir.dt.int16

_PROGRAM_CACHE = {}


def _ap(base, extra_offset, pattern):
    return bass.AP(base.tensor, base.offset + extra_offset, pattern)


def build_program(cap):
    """SPMD Bass program for `cap` boxes per core (cap % 16 == 0)."""
    capo = cap // 8  # 8-box groups, even
    nc = bacc.Bacc("TRN2", target_bir_lowering=False, debug=False)

    NE = HW + 1  # gather num_elems (zeroed pad pair at index HW)
    img_d = nc.dram_tensor("img", [257 * HW], F32, kind="ExternalInput")
    widx_d = nc.dram_tensor("widx", [128, capo * 196], I16, kind="ExternalInput")
    wx_d = nc.dram_tensor("wx", [128, capo * 224], F16, kind="ExternalInput")
    wy_d = nc.dram_tensor("wy", [128, capo * 112], F16, kind="ExternalInput")
    out_d = nc.dram_tensor(
        "out", [2, capo // 2, 128, 2 * 1568], F16, kind="ExternalOutput"
    )

    A = mybir.AluOpType
    ADD, SUB, MUL = A.add, A.subtract, A.mult

    with tile.TileContext(nc) as tc:
        with (
            tc.tile_pool(name="big", bufs=1) as bigp,
            tc.tile_pool(name="chk", bufs=2) as chkp,
            tc.tile_pool(name="gtp", bufs=2) as gtp,
            tc.tile_pool(name="ppp", bufs=2) as ppp,
            tc.tile_pool(name="stg", bufs=2) as stgp,
        ):
            widx = bigp.tile([128, capo * 196], I16, tag="widx")
            nc.sync.dma_start(widx[:], widx_d[:])
            wx = bigp.tile([128, capo * 224], F16, tag="wx")
            nc.sync.dma_start(wx[:], wx_d[:])
            wy = bigp.tile([128, capo * 112], F16, tag="wy")
            nc.sync.dma_start(wy[:], wy_d[:])

            imgdup = bigp.tile([128, NE, 2], F16, tag="imgdup")
            nchunks = HW // CK
            CK2 = CK // 2
            for h in range(2):
                for k in range(nchunks):
                    ch = chkp.tile([128, CK + 1], F32, tag="gch")
                    off = (h * 128) * HW + k * CK
                    nc.sync.dma_start(
                        ch[:], _ap(img_d[:], off, [[HW, 128], [1, CK + 1]])
                    )
                    o = k * CK
                    # dup-pair conversion split across Scalar and Vector
                    nc.scalar.copy(imgdup[:, o : o + CK2, 0], ch[:, 0:CK2])
                    nc.scalar.copy(imgdup[:, o : o + CK2, 1], ch[:, 1 : CK2 + 1])
                    nc.vector.tensor_copy(
                        out=imgdup[:, o + CK2 : o + CK, 0], in_=ch[:, CK2:CK]
                    )
                    nc.vector.tensor_copy(
                        out=imgdup[:, o + CK2 : o + CK, 1], in_=ch[:, CK2 + 1 : CK + 1]
                    )
                nc.vector.memset(imgdup[:, HW : HW + 1, :], 0.0)

                stgt = None
                for cc in range(capo):
                    gt = gtp.tile([128, 3136, 2], F16, tag="gt")
                    nc.gpsimd.ap_gather(
                        gt[:],
                        imgdup[:],
                        widx[:, cc * 196 : (cc + 1) * 196],
                        channels=128,
                        num_elems=NE,
                        d=2,
                        num_idxs=3136,
                    )
                    # x-lerp: one in-place MUL by interleaved (1-xl, xl).
                    # gt free elems: (j:448, i:32, tb:16, n:2, lr:1); dims
                    # collapse to [j, (i,tb), (n,lr)] = [14, 28, 16].
                    gt_ap = _ap(gt[:], 0, [gt[:].ap[0], [448, 14], [16, 28], [1, 16]])
                    wx_ap = _ap(
                        wx[:], cc * 224, [wx[:].ap[0], [16, 14], [0, 28], [1, 16]]
                    )
                    nc.vector.tensor_tensor(out=gt_ap, in0=gt_ap, in1=wx_ap, op=MUL)
                    # pair-sum (l + r) -> pp[q], q = (j, i, tb, n)
                    pp = ppp.tile([128, 3136], F16, tag="pp")
                    l_v = _ap(gt[:], 0, [gt[:].ap[0], [2, 3136]])
                    r_v = _ap(gt[:], 1, [gt[:].ap[0], [2, 3136]])
                    nc.vector.tensor_tensor(out=pp[:], in0=l_v, in1=r_v, op=ADD)
                    # y-lerp: d2 = (bot - top) * wy; out = top + d2
                    d2 = ppp.tile([128, 1568], F16, tag="d2")
                    t_v = _ap(pp[:], 0, [pp[:].ap[0], [16, 196], [1, 8]])
                    b_v = _ap(pp[:], 8, [pp[:].ap[0], [16, 196], [1, 8]])
                    nc.vector.tensor_tensor(out=d2[:], in0=b_v, in1=t_v, op=SUB)
                    d2_v = _ap(d2[:], 0, [d2[:].ap[0], [112, 14], [8, 14], [1, 8]])
                    wy_ap = _ap(
                        wy[:], cc * 112, [wy[:].ap[0], [0, 14], [8, 14], [1, 8]]
                    )
                    nc.vector.tensor_tensor(out=d2_v, in0=d2_v, in1=wy_ap, op=MUL)
                    if cc % 2 == 0:
                        stgt = stgp.tile([128, 2, 1568], F16, tag="stg")
                    t3_v = _ap(pp[:], 0, [pp[:].ap[0], [224, 14], [16, 14], [1, 8]])
                    d3_v = _ap(d2[:], 0, [d2[:].ap[0], [112, 14], [8, 14], [1, 8]])
                    so = stgt[:, cc % 2, :]
                    so_v = _ap(so, 0, [so.ap[0], [112, 14], [8, 14], [1, 8]])
                    nc.vector.tensor_tensor(out=so_v, in0=t3_v, in1=d3_v, op=ADD)
                    if cc % 2 == 1:
                        dst = out_d[h, cc // 2]
                        nc.sync.dma_start(
                            dst, stgt[:].rearrange("p a b -> p (a b)")
                        )

    nc.compile()
    return nc


def _host_tables(bk, capo):
    """Gather indices + bilinear weights for one core's padded boxes.

    bk: [cap, 4] fp32 (y1, x1, y2, x2), cap = capo * 8.
    Returns widx [128, capo*196] i16, wx [128, capo*224] f16,
    wy [128, capo*112] f16 (rows replicated as ap_gather expects).
    """
    cap = capo * 8
    f = np.float32
    iota = np.arange(CH, dtype=f)
    y1, x1, y2, x2 = bk[:, 0], bk[:, 1], bk[:, 2], bk[:, 3]

    def axis(lo, hi):
        scale = (hi - lo) * f(H - 1) / f(CH - 1)
        inv = lo[:, None] * f(H - 1) + iota[None, :] * scale[:, None]  # [cap,14]
        valid = (inv >= f(0)) & (inv <= f(H - 1))
        fl = np.floor(inv)
        frac = (inv - fl).astype(f)
        lo_i = np.clip(fl, 0, H - 1).astype(np.int32)
        hi_i = np.clip(fl + 1, 0, H - 1).astype(np.int32)
        return valid, frac, lo_i, hi_i

    vy, yl, ti, bi = axis(y1, y2)
    vx, xl, li, _ri = axis(x1, x2)

    # idx[n, tb, i, j] = row*W + col, or HW when masked
    rows = np.stack([ti, bi], axis=1)  # [cap, 2, 14] (i)
    idx = rows[:, :, :, None] * W + li[:, None, None, :]  # [cap,2,14i,14j]
    valid = (vy[:, None, :, None] & vx[:, None, None, :])  # [cap,2,14i,14j]
    idx = np.where(valid, idx, HW).astype(np.int16)

    # widx[w=(tb,n), cc, j, i] with w the 16-row wrap pattern
    idx = idx.reshape(capo, 8, 2, CH, CW)  # [cc, n, tb, i, j]
    wi = idx.transpose(2, 1, 0, 4, 3)  # [tb, n, cc, j, i]
    wi = wi.reshape(16, capo * 196)
    widx = np.tile(wi, (8, 1))  # [128, capo*196]

    # wx[cc, j, n, lr] = (1-xl, xl); wy[cc, i, n] = yl
    xl = xl.reshape(capo, 8, CW)  # [cc, n, j]
    wxa = np.empty((capo, CW, 8, 2), np.float16)
    wxa[..., 0] = (f(1) - xl).transpose(0, 2, 1)
    wxa[..., 1] = xl.transpose(0, 2, 1)
    wx = np.broadcast_to(wxa.reshape(1, capo * 224), (128, capo * 224))

    yl = yl.reshape(capo, 8, CH).transpose(0, 2, 1)  # [cc, i, n]
    wy = np.broadcast_to(
        yl.astype(np.float16).reshape(1, capo * 112), (128, capo * 112)
    )
    return widx, np.ascontiguousarray(wx), np.ascontiguousarray(wy)


def make_in_maps(image, boxes, box_ind):
    image = np.asarray(image, dtype=np.float32)
    boxes = np.asarray(boxes, dtype=np.float32)
    box_ind = np.asarray(box_ind, dtype=np.int32)

    order = np.argsort(box_ind, kind="stable")
    counts = np.bincount(box_ind, minlength=N_CORES)
    cap = max(16, int(math.ceil(counts.max() / 16.0)) * 16)
    capo = cap // 8
    starts = np.zeros(N_CORES + 1, np.int64)
    starts[1:] = np.cumsum(counts)

    in_maps = []
    for k in range(N_CORES):
        img_k = np.empty(257 * HW, np.float32)
        img_k[: 256 * HW] = image[k].reshape(-1)
        img_k[256 * HW :] = 0.0
        bk = np.zeros((cap, 4), np.float32)
        sel = order[starts[k] : starts[k + 1]]
        bk[: counts[k]] = boxes[sel]
        widx, wx, wy = _host_tables(bk, capo)
        in_maps.append({"img": img_k, "widx": widx, "wx": wx, "wy": wy})
    return in_maps, order, counts, starts, cap


def kernel(image, boxes, box_ind):
    in_maps, order, counts, starts, cap = make_in_maps(image, boxes, box_ind)

    nc = _PROGRAM_CACHE.get(cap)
    if nc is None:
        nc = build_program(cap)
        _PROGRAM_CACHE[cap] = nc

    res = run_bass_kernel_spmd(nc, in_maps, core_ids=list(range(N_CORES)))

    n = boxes.shape[0]
    capo = cap // 8
    out = np.empty((n, C, CH, CW), np.float32)
    for k in range(N_CORES):
        sel = order[starts[k] : starts[k + 1]]
        arr = res.results[k]["out"]  # [2, capo//2, 128, 3136] f16
        arr = arr.reshape(2, capo // 2, 128, 2, CW, CH, 8)  # h,cp,c,sub,j,i,n
        arr = arr.transpose(1, 3, 6, 0, 2, 5, 4)  # cp,sub,n,h,c,i,j
        ok = arr.reshape(cap, C, CH, CW)[: counts[k]].astype(np.float32)
        out[sel] = ok
    return out
